# revision 1
# baseline (speedup 1.0000x reference)
"""Trainium2 Bass kernel: top-2 MoE (8 experts, E=1024, H=1536, T=16384).

Sharding: data-parallel over the batch axis -- each of the 8 NeuronCores
processes one batch row (2048 tokens) end to end:
  1. fp32 router on device (logits matmul, softmax, top-2 via threshold mask)
  2. on-device stream compaction (gpsimd sparse_gather) -> per-expert token
     lists in the 16-wrapped int16 format the custom DMA ops consume
  3. dma_gather(transpose=True) pulls each expert's token rows from HBM in
     bf16, already transposed to feature-major for the matmuls
  4. per-expert FFN at a static capacity of 640 tokens (actual max per-expert
     count for the routed input is checked on host):
     H^T = gelu(W1^T X^T + b1); then token-major Y via stationary H^T tiles
  5. gating (softmax prob of the selected expert) applied as a per-partition
     ACT scale while evacuating PSUM
  6. dma_scatter_add accumulates gated rows into the fp32 output (the
     ExternalOutput buffer is pre-zeroed by the runtime)

Host work is limited to sharding/staging (slice, transpose, bf16 cast of the
staged copies) and a capacity-safety check; all arithmetic producing the
output runs on the NeuronCores.
"""

import numpy as np
import ml_dtypes

import concourse.bacc as bacc
import concourse.mybir as mybir
import concourse.tile as tile
from concourse.alu_op_type import AluOpType
from concourse.bass_utils import run_bass_kernel_spmd

F32 = mybir.dt.float32
BF16 = mybir.dt.bfloat16
I16 = mybir.dt.int16
U32 = mybir.dt.uint32
AF = mybir.ActivationFunctionType

B, N, E, H, NE = 8, 2048, 1024, 1536, 8
KT = E // 128          # 8 k-tiles of x features
HT = H // 128          # 12 tiles of hidden
C = 640                # per-expert token capacity (multiple of 128)
CT = C // 128          # 5 token tiles per expert
CW = C // 16           # wrapped idx columns
NP = N + 128           # gather/scatter tables padded with a zero dummy row
SGF = 128 + CW         # sparse_gather free dim: 2048 real slots + C dummies

_CACHE = {}


def _build_nc():
    nc = bacc.Bacc("TRN2", target_bir_lowering=False)

    xT = nc.dram_tensor("xT", [E, N], F32, kind="ExternalInput")
    xbf = nc.dram_tensor("xbf", [NP, E], BF16, kind="ExternalInput")
    wr = nc.dram_tensor("wr", [E, NE], F32, kind="ExternalInput")
    w1 = nc.dram_tensor("w1", [NE, E, H], BF16, kind="ExternalInput")
    w2 = nc.dram_tensor("w2", [NE, H, E], BF16, kind="ExternalInput")
    tok1 = nc.dram_tensor("tok1", [128, 16, 1], F32, kind="ExternalInput")
    eye8 = nc.dram_tensor("eye8", [8, 8], F32, kind="ExternalInput")
    brv = nc.dram_tensor("brv", [8, 1], F32, kind="ExternalInput")
    b1v = nc.dram_tensor("b1v", [128, NE, HT], F32, kind="ExternalInput")
    out = nc.dram_tensor("out", [NP, E], F32, kind="ExternalOutput")

    midx_d = nc.dram_tensor("midx_d", [NE, N], F32)
    lists_d = nc.dram_tensor("lists_d", [NE, 16, CW], I16)
    gat_d = nc.dram_tensor("gat_d", [NP, 64], F32)

    with tile.TileContext(nc) as tc:
        with (
            tc.tile_pool(name="consts", bufs=1) as cpool,
            tc.tile_pool(name="lists", bufs=NE) as lpool,
            tc.tile_pool(name="xg", bufs=2) as xg_pool,
            tc.tile_pool(name="gt", bufs=2) as gt_pool,
            tc.tile_pool(name="w1p", bufs=2) as w1_pool,
            tc.tile_pool(name="w2p", bufs=2) as w2_pool,
            tc.tile_pool(name="hT", bufs=1) as h_pool,
            tc.tile_pool(name="y", bufs=1) as y_pool,
            tc.tile_pool(name="psH", bufs=2, space="PSUM") as psH_pool,
            tc.tile_pool(name="psY", bufs=2, space="PSUM") as psY_pool,
        ):
            # ---- constants ----
            wr_sb = cpool.tile([128, KT, NE], F32)
            nc.sync.dma_start(wr_sb[:], wr.rearrange("(k p) c -> p k c", p=128))
            eye_sb = cpool.tile([8, 8], F32)
            nc.sync.dma_start(eye_sb[:], eye8[:])
            tok1_sb = cpool.tile([128, 16, 1], F32)
            nc.sync.dma_start(tok1_sb[:], tok1[:])
            brv_sb = cpool.tile([8, 1], F32)
            nc.sync.dma_start(brv_sb[:], brv[:])
            b1_sb = cpool.tile([128, NE, HT], F32)
            nc.sync.dma_start(b1_sb[:], b1v[:])

            rpool_cm = tc.tile_pool(name="router_sb", bufs=1)
            xt_pool_cm = tc.tile_pool(name="router_x", bufs=2)
            with rpool_cm as rpool, xt_pool_cm as xt_pool:
                # ---- router: logits^T [8, N] = Wr^T @ X^T (+ br), fp32 ----
                ltr = rpool.tile([8, N], F32)
                with tc.tile_pool(name="router_ps", bufs=1, space="PSUM") as psL_pool:
                    psL = [psL_pool.tile([8, 512], F32, tag=f"psL{i}",
                                         name=f"psL{i}")
                           for i in range(4)]
                    for k in range(KT):
                        xt_sb = xt_pool.tile([128, N], F32)
                        nc.sync.dma_start(xt_sb[:], xT[128 * k:128 * (k + 1), :])
                        for c4 in range(4):
                            nc.tensor.matmul(
                                psL[c4][:],
                                lhsT=wr_sb[:, k, :],
                                rhs=xt_sb[:, 512 * c4:512 * (c4 + 1)],
                                start=(k == 0),
                                stop=(k == KT - 1),
                            )
                    for c4 in range(4):
                        nc.scalar.activation(
                            ltr[:, 512 * c4:512 * (c4 + 1)], psL[c4][:],
                            AF.Identity, bias=brv_sb[:],
                        )

                # ---- transpose logits to token-major [128, 16*8] ----
                ltm = rpool.tile([128, 16, NE], F32)
                with tc.tile_pool(name="psT", bufs=1, space="PSUM") as psT_pool:
                    psT = psT_pool.tile([128, 128], F32)
                    for bi in range(16):
                        nc.tensor.transpose(
                            out=psT[:, 8 * bi:8 * (bi + 1)],
                            in_=ltr[:, 128 * bi:128 * (bi + 1)],
                            identity=eye_sb[:],
                        )
                    nc.vector.tensor_copy(ltm[:], psT[:])

                # ---- top-2 selection on raw fp32 logits (keeps the exp LUT
                # out of the selection path; softmax is monotone so top-2 by
                # logits == top-2 by probs) ----
                rmax = rpool.tile([128, 16, 1], F32)
                nc.vector.tensor_reduce(rmax[:], ltm[:], axis=mybir.AxisListType.X,
                                        op=AluOpType.max)
                ismax = rpool.tile([128, 16, NE], F32)
                nc.vector.tensor_tensor(ismax[:], ltm[:],
                                        rmax[:].to_broadcast([128, 16, NE]),
                                        op=AluOpType.is_ge)
                masked2 = rpool.tile([128, 16, NE], F32)
                nc.vector.scalar_tensor_tensor(masked2[:], in0=ismax[:],
                                               scalar=-1.0e5, in1=ltm[:],
                                               op0=AluOpType.mult,
                                               op1=AluOpType.add)
                thr = rpool.tile([128, 16, 1], F32)
                nc.vector.tensor_reduce(thr[:], masked2[:],
                                        axis=mybir.AxisListType.X,
                                        op=AluOpType.max)
                mask = rpool.tile([128, 16, NE], F32)
                nc.vector.tensor_tensor(mask[:], ltm[:],
                                        thr[:].to_broadcast([128, 16, NE]),
                                        op=AluOpType.is_ge)

                # ---- softmax probs (gating values only) ----
                cmb = rpool.tile([128, 16, NE], F32)
                nc.vector.tensor_sub(cmb[:], ltm[:],
                                     rmax[:].to_broadcast([128, 16, NE]))
                nc.scalar.activation(cmb[:], cmb[:], AF.Exp)
                esum = rpool.tile([128, 16, 1], F32)
                nc.vector.tensor_reduce(esum[:], cmb[:], axis=mybir.AxisListType.X,
                                        op=AluOpType.add)
                rs = rpool.tile([128, 16, 1], F32)
                nc.vector.reciprocal(rs[:], esum[:])
                nc.vector.tensor_tensor(cmb[:], cmb[:],
                                        rs[:].to_broadcast([128, 16, NE]),
                                        op=AluOpType.mult)
                midx = rpool.tile([128, 16, NE], F32)
                nc.vector.tensor_tensor(midx[:], mask[:],
                                        tok1_sb[:].to_broadcast([128, 16, NE]),
                                        op=AluOpType.mult)
                nc.vector.tensor_scalar_add(midx[:], midx[:], -1.0)

                # gating table (token rows zero-padded to 64 floats so
                # dma_gather's 256B-aligned rows stay fully initialized)
                cmb64 = rpool.tile([128, 16, 64], F32)
                nc.vector.memset(cmb64[:], 0.0)
                nc.vector.tensor_copy(cmb64[:, :, 0:NE], cmb[:])
                nc.sync.dma_start(
                    gat_d[0:N].rearrange("(bi p) c -> p bi c", p=128), cmb64[:])
                zrow = rpool.tile([128, 64], F32)
                nc.vector.memset(zrow[:], 0.0)
                nc.sync.dma_start(gat_d[N:NP, :], zrow[:])
                # masked token-id planes, one per expert
                for e in range(NE):
                    nc.sync.dma_start(
                        midx_d[e].rearrange("(bi p) -> p bi", p=128), midx[:, :, e])

            # ---- per-expert compaction (sparse_gather ucode library) ----
            # Per-expert compaction. HW sparse_gather writes garbage beyond
            # num_found, so instead of trusting the tail we append C dummy
            # slots (value N = dummy token) to the *input*: the compacted
            # output then always starts with the real tokens followed by
            # dummies, making the first C slots deterministic and every idx
            # list exactly C valid entries (constant-count custom DMAs).
            idx_sbs = []
            for e in range(NE):
                sg_in = lpool.tile([16, SGF], F32, tag="sg_in", bufs=2)
                nc.vector.memset(sg_in[:], float(N))
                nc.sync.dma_start(sg_in[:, 0:128],
                                  midx_d[e].rearrange("(p f) -> p f", p=16))
                slist = lpool.tile([16, SGF], F32, tag="slist", bufs=2)
                nfound = lpool.tile([1, 1], U32, tag="nfound", bufs=2)
                nc.gpsimd.sparse_gather(slist[:], sg_in[:], num_found=nfound[:])
                ilist = lpool.tile([16, CW], I16, tag="ilist", bufs=2)
                nc.vector.tensor_copy(ilist[:], slist[:, 0:CW])
                nc.sync.dma_start(lists_d[e], ilist[:])
                idx_sb = lpool.tile([128, CW], I16, tag="idx")
                for g in range(8):
                    nc.sync.dma_start(idx_sb[16 * g:16 * (g + 1), :], lists_d[e])
                idx_sbs.append(idx_sb)

            # ---- per-expert FFN (mlp library: dma_gather / dma_scatter_add) ----
            for e in range(NE):
                xg = xg_pool.tile([128, KT, C], BF16)
                nc.gpsimd.dma_gather(
                    out_ap=xg[:], in_ap=xbf[:], idxs_ap=idx_sbs[e][:],
                    num_idxs=C, num_idxs_reg=C, elem_size=E, transpose=True)
                gt = gt_pool.tile([128, CT, 64], F32)
                nc.gpsimd.dma_gather(
                    out_ap=gt[:], in_ap=gat_d[:], idxs_ap=idx_sbs[e][:],
                    num_idxs=C, num_idxs_reg=C, elem_size=64, transpose=False)

                w1_sb = w1_pool.tile([128, KT, H], BF16)
                nc.sync.dma_start(w1_sb[:], w1[e].rearrange("(k p) h -> p k h", p=128))
                w2_sb = w2_pool.tile([128, HT, E], BF16)
                nc.sync.dma_start(w2_sb[:], w2[e].rearrange("(k p) f -> p k f", p=128))

                hT = h_pool.tile([128, HT, C], BF16)
                for h in range(HT):
                    for c0, cw in ((0, 512), (512, 128)):
                        ps = psH_pool.tile([128, cw], F32, tag="psH")
                        for k in range(KT):
                            nc.tensor.matmul(
                                ps[:], lhsT=w1_sb[:, k, 128 * h:128 * (h + 1)],
                                rhs=xg[:, k, c0:c0 + cw],
                                start=(k == 0), stop=(k == KT - 1))
                        nc.scalar.activation(hT[:, h, c0:c0 + cw], ps[:],
                                             AF.Gelu, bias=b1_sb[:, e, h:h + 1])

                y_sb = y_pool.tile([128, CT, E], F32)
                for tt in range(CT):
                    for n2 in range(2):
                        ps = psY_pool.tile([128, 512], F32, tag="psY")
                        for k2 in range(HT):
                            nc.tensor.matmul(
                                ps[:], lhsT=hT[:, k2, 128 * tt:128 * (tt + 1)],
                                rhs=w2_sb[:, k2, 512 * n2:512 * (n2 + 1)],
                                start=(k2 == 0), stop=(k2 == HT - 1))
                        nc.scalar.activation(
                            y_sb[:, tt, 512 * n2:512 * (n2 + 1)], ps[:],
                            AF.Copy, scale=gt[:, tt, e:e + 1])

                nc.gpsimd.dma_scatter_add(
                    out_ap=out[:], in_ap=y_sb[:], idxs_ap=idx_sbs[e][:],
                    num_idxs=C, num_idxs_reg=C, elem_size=E)

    return nc


def get_nc():
    if "nc" not in _CACHE:
        nc = _build_nc()
        nc.finalize()  # Bacc.compile(): reg alloc, library-load insertion, ...
        _CACHE["nc"] = nc
    return _CACHE["nc"]


def make_in_maps(inputs):
    x = np.asarray(inputs["x"], dtype=np.float32)
    Wr = np.asarray(inputs["Wr"], dtype=np.float32)
    br = np.asarray(inputs["br"], dtype=np.float32)
    W1 = np.asarray(inputs["W1"], dtype=np.float32)
    b1 = np.asarray(inputs["b1"], dtype=np.float32)
    W2 = np.asarray(inputs["W2"], dtype=np.float32)
    b2 = np.asarray(inputs["b2"], dtype=np.float32)
    assert x.shape == (B, N, E) and W1.shape == (NE, E, H) and W2.shape == (NE, H, E)
    if b2.any():
        raise NotImplementedError("nonzero b2 path not emitted in this kernel")

    # capacity guard: the kernel is compiled for a static per-expert capacity
    # of C tokens per core; verify the actual routing fits.
    logits = x.reshape(B * N, E) @ Wr + br
    part = np.partition(logits, NE - 2, axis=-1)[:, NE - 2:NE - 1]
    sel = logits >= part
    counts = sel.reshape(B, N, NE).sum(1)
    if counts.max() > C:
        raise RuntimeError(f"expert capacity exceeded: {counts.max()} > {C}")

    bf = ml_dtypes.bfloat16
    tok1 = (np.arange(16)[None, :] * 128 + np.arange(128)[:, None] + 1.0)
    tok1 = tok1.astype(np.float32).reshape(128, 16, 1)
    eye8 = np.eye(8, dtype=np.float32)
    brv = br.reshape(NE, 1).astype(np.float32)
    # b1v[p, e, h] = b1[e, h*128 + p]
    b1v = np.ascontiguousarray(b1.reshape(NE, HT, 128).transpose(2, 0, 1))
    W1b = W1.astype(bf)
    W2b = W2.astype(bf)

    in_maps = []
    for c in range(B):
        in_maps.append({
            "xT": np.ascontiguousarray(x[c].T),
            "xbf": np.concatenate(
                [x[c], np.zeros((NP - N, E), np.float32)], axis=0).astype(bf),
            "wr": Wr,
            "w1": W1b,
            "w2": W2b,
            "tok1": tok1,
            "eye8": eye8,
            "brv": brv,
            "b1v": b1v,
        })
    return in_maps


def run(inputs, **kw):
    in_maps = make_in_maps(inputs)
    nc = get_nc()
    res = run_bass_kernel_spmd(nc, in_maps, list(range(B)), **kw)
    out = np.stack([res.results[c]["out"][0:N] for c in range(B)], axis=0)
    return out.astype(np.float32), res


def kernel(**inputs):
    out, _ = run(inputs)
    return out



# revision 20
# speedup vs baseline: 1.2795x; 1.2795x over previous
"""Trainium2 Bass kernel: top-2 MoE (8 experts, E=1024, H=1536, T=16384).

Sharding: data-parallel over the batch axis -- each of the 8 NeuronCores
processes one batch row (2048 tokens) end to end:
  1. fp32 router on device (logits matmul, softmax, top-2 via threshold mask)
  2. on-device compaction: the masked token-id planes are transposed on the
     tensor engine into per-expert [16,128] rows and compacted with the
     gpsimd sparse_gather ucode -> int16 idx lists (trailing -1 padding,
     which the gather/scatter ucode trims)
  3. dma_gather(transpose=True) pulls each expert's token rows from an
     SBUF-resident striped copy of x (bf16), already feature-major
  4. per-expert FFN at a static capacity of 640 tokens (actual max per-expert
     count for the routed input is checked on host):
     H^T = gelu(W1^T X^T + b1); then token-major Y via stationary H^T tiles
  5. gating (softmax prob of the selected expert) applied as a per-partition
     ACT scale while evacuating PSUM; y stored bf16
  6. dma_scatter_add accumulates gated bf16 rows into the bf16 output (the
     ExternalOutput buffer is pre-zeroed by the runtime); -1 idx tails are
     skipped by the ucode so only real tokens are written

All heavy DMAs are host-prestaged to be fully contiguous per partition
(weights 24KB/partition/expert-matrix, xT 8KB/partition/tile, xbf one 4.2MB
transfer).  Host work is limited to sharding/staging (slice, transpose, bf16
cast of the staged copies), a capacity-safety check, and undoing the row
permutation of the output; all arithmetic producing the output runs on the
NeuronCores.  Output rows use the p-major id convention id = (t%128)*16 +
t//128 so the on-chip gating table can be written contiguously.
"""

import numpy as np
import ml_dtypes

import concourse.bacc as bacc
import concourse.mybir as mybir
import concourse.tile as tile
from concourse.alu_op_type import AluOpType
from concourse.bass_utils import run_bass_kernel_spmd

F32 = mybir.dt.float32
BF16 = mybir.dt.bfloat16
I16 = mybir.dt.int16
U32 = mybir.dt.uint32
AF = mybir.ActivationFunctionType

B, N, E, H, NE = 8, 2048, 1024, 1536, 8
KT = E // 128          # 8 k-tiles of x features
HT = H // 128          # 12 tiles of hidden
C = 640                # per-expert token capacity (multiple of 128)
CT = C // 128          # 5 token tiles per expert
CW = C // 16           # wrapped idx columns
SGF = 128 + CW         # sparse_gather free dim: 2048 real slots + C dummies
NP = N + 128           # gather/scatter tables padded with zero dummy rows

_CACHE = {}


def _build_nc(counts=None, sim=False):
    nir = (lambda e: C) if counts is None else (lambda e: int(counts[e]))
    gelu_af = AF.Identity if sim else AF.Gelu
    nc = bacc.Bacc("TRN2", target_bir_lowering=False)

    xT = nc.dram_tensor("xT", [128, KT * N], F32, kind="ExternalInput")
    xbf = nc.dram_tensor("xbf", [NP, E], BF16, kind="ExternalInput")
    wr = nc.dram_tensor("wr", [128, KT * NE], F32, kind="ExternalInput")
    # weights staged in halves (W1 by hidden-tile range, W2 by output-column
    # half) so each half double-buffers in a 3-deep rotation: 36KB/partition
    # instead of 48 for a 2-deep rotation of full matrices.
    w1 = nc.dram_tensor("w1", [NE, 2, 128, KT * (H // 2)], BF16,
                        kind="ExternalInput")
    w2 = nc.dram_tensor("w2", [NE, 2, 128, HT * (E // 2)], BF16,
                        kind="ExternalInput")
    tok1 = nc.dram_tensor("tok1", [128, 16, 1], F32, kind="ExternalInput")
    eye128 = nc.dram_tensor("eye128", [128, 128], F32, kind="ExternalInput")
    brv = nc.dram_tensor("brv", [8, 1], F32, kind="ExternalInput")
    b1v = nc.dram_tensor("b1v", [128, NE, HT], F32, kind="ExternalInput")
    out = nc.dram_tensor("out", [NP, E], BF16, kind="ExternalOutput")

    gat_d = nc.dram_tensor("gat_d", [NP, 64], F32)
    lists_d = nc.dram_tensor("lists_d", [NE, 16, CW], I16)

    with tile.TileContext(nc) as tc:
        with (
            tc.tile_pool(name="consts", bufs=1) as cpool,
            tc.tile_pool(name="lists", bufs=NE) as lpool,
            tc.tile_pool(name="xg", bufs=2) as xg_pool,
            tc.tile_pool(name="gt", bufs=2) as gt_pool,
            tc.tile_pool(name="w1p", bufs=3) as w1_pool,
            tc.tile_pool(name="w2p", bufs=3) as w2_pool,
            tc.tile_pool(name="hT", bufs=1) as h_pool,
            tc.tile_pool(name="y", bufs=1) as y_pool,
            tc.tile_pool(name="psH", bufs=2, space="PSUM") as psH_pool,
            tc.tile_pool(name="psY", bufs=2, space="PSUM") as psY_pool,
        ):
            # ---- constants (small, issued first) ----
            wr_sb = cpool.tile([128, KT, NE], F32)
            nc.sync.dma_start(wr_sb[:], wr[:])
            eye_sb = cpool.tile([128, 128], F32)
            nc.sync.dma_start(eye_sb[:], eye128[:])
            tok1_sb = cpool.tile([128, 16, 1], F32)
            nc.sync.dma_start(tok1_sb[:], tok1[:])
            brv_sb = cpool.tile([8, 1], F32)
            nc.sync.dma_start(brv_sb[:], brv[:])
            b1_sb = cpool.tile([128, NE, HT], F32)
            nc.sync.dma_start(b1_sb[:], b1v[:])

            rpool_cm = tc.tile_pool(name="router_sb", bufs=1)
            xt_pool_cm = tc.tile_pool(name="router_x", bufs=3)
            idx_sbs = []
            w_tiles = {}

            def load_w(e, half):
                w1_sb = w1_pool.tile([128, KT, H // 2], BF16, tag="w1",
                                     name=f"w1sb{e}_{half}")
                nc.sync.dma_start(w1_sb[:], w1[e, half])
                w2_sb = w2_pool.tile([128, HT, E // 2], BF16, tag="w2",
                                     name=f"w2sb{e}_{half}")
                nc.sync.dma_start(w2_sb[:], w2[e, half])
                w_tiles[(e, half)] = (w1_sb, w2_sb)

            with rpool_cm as rpool, xt_pool_cm as xt_pool:
                # ---- router: logits^T [8, N] = Wr^T @ X^T (+ br), fp32 ----
                ltr = rpool.tile([8, N], F32)
                with tc.tile_pool(name="router_ps", bufs=1, space="PSUM") as psL_pool:
                    psL = [psL_pool.tile([8, 512], F32, tag=f"psL{i}",
                                         name=f"psL{i}")
                           for i in range(4)]
                    for k in range(KT):
                        xt_sb = xt_pool.tile([128, N], F32, tag="xt", bufs=3)
                        nc.sync.dma_start(xt_sb[:], xT[:, N * k:N * (k + 1)])
                        for c4 in range(4):
                            nc.tensor.matmul(
                                psL[c4][:],
                                lhsT=wr_sb[:, k, :],
                                rhs=xt_sb[:, 512 * c4:512 * (c4 + 1)],
                                start=(k == 0),
                                stop=(k == KT - 1),
                            )
                    for c4 in range(4):
                        nc.scalar.activation(
                            ltr[:, 512 * c4:512 * (c4 + 1)], psL[c4][:],
                            AF.Identity, bias=brv_sb[:],
                        )

                # ---- xbf (gather source) load + weight prefetch: issued
                # here so their transfers queue right behind the xT tiles ----
                load_w(0, 0)
                load_w(0, 1)
                load_w(1, 0)

                # ---- transpose logits to token-major [128, 16*8] ----
                ltm = rpool.tile([128, 16, NE], F32)
                with tc.tile_pool(name="psT", bufs=1, space="PSUM") as psT_pool:
                    psT = psT_pool.tile([128, 128], F32)
                    for bi in range(16):
                        nc.tensor.transpose(
                            out=psT[:, 8 * bi:8 * (bi + 1)],
                            in_=ltr[:, 128 * bi:128 * (bi + 1)],
                            identity=eye_sb[0:8, 0:8],
                        )
                    nc.vector.tensor_copy(ltm[:], psT[:])

                    # ---- top-2 selection on raw fp32 logits (softmax is
                    # monotone so top-2 by logits == top-2 by probs) ----
                    rmax = rpool.tile([128, 16, 1], F32)
                    nc.vector.tensor_reduce(rmax[:], ltm[:],
                                            axis=mybir.AxisListType.X,
                                            op=AluOpType.max)
                    ismax = rpool.tile([128, 16, NE], F32)
                    nc.vector.tensor_tensor(ismax[:], ltm[:],
                                            rmax[:].to_broadcast([128, 16, NE]),
                                            op=AluOpType.is_ge)
                    masked2 = rpool.tile([128, 16, NE], F32)
                    nc.vector.scalar_tensor_tensor(masked2[:], in0=ismax[:],
                                                   scalar=-1.0e5, in1=ltm[:],
                                                   op0=AluOpType.mult,
                                                   op1=AluOpType.add)
                    thr = rpool.tile([128, 16, 1], F32)
                    nc.vector.tensor_reduce(thr[:], masked2[:],
                                            axis=mybir.AxisListType.X,
                                            op=AluOpType.max)
                    mask = rpool.tile([128, 16, NE], F32)
                    nc.vector.tensor_tensor(mask[:], ltm[:],
                                            thr[:].to_broadcast([128, 16, NE]),
                                            op=AluOpType.is_ge)

                    # ---- softmax probs (gating values only) ----
                    cmb = rpool.tile([128, 16, NE], F32)
                    nc.vector.tensor_sub(cmb[:], ltm[:],
                                         rmax[:].to_broadcast([128, 16, NE]))
                    nc.scalar.activation(cmb[:], cmb[:], AF.Exp)
                    esum = rpool.tile([128, 16, 1], F32)
                    nc.vector.tensor_reduce(esum[:], cmb[:],
                                            axis=mybir.AxisListType.X,
                                            op=AluOpType.add)
                    rs = rpool.tile([128, 16, 1], F32)
                    nc.vector.reciprocal(rs[:], esum[:])
                    nc.vector.tensor_tensor(cmb[:], cmb[:],
                                            rs[:].to_broadcast([128, 16, NE]),
                                            op=AluOpType.mult)
                    # masked token ids: id+1 if selected else 0, minus 1
                    midx = rpool.tile([128, 16, NE], F32)
                    nc.vector.tensor_tensor(midx[:], mask[:],
                                            tok1_sb[:].to_broadcast([128, 16, NE]),
                                            op=AluOpType.mult)
                    nc.vector.tensor_scalar_add(midx[:], midx[:], -1.0)

                    # gating table (rows zero-padded to 64 floats = 256B so
                    # dma_gather's 256B-aligned rows stay fully initialized);
                    # row order is p-major (id = p*16 + bi) so the write is
                    # fully contiguous: partition p -> rows p*16..p*16+15.
                    cmb64 = rpool.tile([128, 16, 64], F32)
                    nc.vector.memset(cmb64[:], 0.0)
                    nc.vector.tensor_copy(cmb64[:, :, 0:NE], cmb[:])
                    nc.sync.dma_start(
                        gat_d[0:N].rearrange("(p b) c -> p b c", p=128),
                        cmb64[:])
                    zrow = rpool.tile([128, 64], F32)
                    nc.vector.memset(zrow[:], 0.0)
                    nc.sync.dma_start(gat_d[N:NP, :], zrow[:])

                    # ---- masked-id planes to expert-major [p, e, bi] so
                    # each expert's plane is a contiguous [128, 16] slice ----
                    midx2 = rpool.tile([128, NE, 16], F32)
                    nc.vector.tensor_copy(
                        midx2[:], midx[:].rearrange("p b e -> p e b"))

                    # ---- per-expert compaction (sparse_gather ucode).  Each
                    # expert's [128, 16] plane is transposed on the tensor
                    # engine into a base-0 [16, 128] tile.  The input is 2048
                    # real slots followed by C dummy slots of value N: the
                    # compacted output then always starts with the real token
                    # ids followed by Ns, making the first C slots
                    # deterministic.  The Ns are then mapped to -1 (the ucode
                    # trims trailing negatives, so gathers/scatters process
                    # only the real tokens). ----
                    for e in range(NE):
                        psE = psT_pool.tile([16, 128], F32, tag="psE",
                                            bufs=2, name=f"psE{e}")
                        nc.tensor.transpose(
                            out=psE[:], in_=midx2[:, e, :],
                            identity=eye_sb[:],
                        )
                        sg_in = lpool.tile([16, SGF], F32, tag="sg_in",
                                           bufs=2, name=f"sg_in{e}")
                        nc.vector.memset(sg_in[:, 128:SGF], float(N))
                        nc.vector.tensor_copy(sg_in[:, 0:128], psE[:])
                        slist = lpool.tile([16, SGF], F32, tag="slist", bufs=2,
                                           name=f"slist{e}")
                        nfound = lpool.tile([1, 1], U32, tag="nfound", bufs=2,
                                            name=f"nfound{e}")
                        nc.gpsimd.sparse_gather(slist[:], sg_in[:],
                                                num_found=nfound[:])
                        ilist = lpool.tile([16, CW], I16, tag="ilist", bufs=2,
                                           name=f"ilist{e}")
                        nc.vector.tensor_copy(ilist[:], slist[:, 0:CW])
                        # replicate the 16-partition list into all 8 gpsimd
                        # core windows via an HBM bounce: SBUF->SBUF DMA here
                        # would deadlock against the XBAR-transpose path the
                        # transposed dma_gather uses.
                        nc.sync.dma_start(lists_d[e], ilist[:])
                        idx_sb = lpool.tile([128, CW], I16, tag="idx",
                                            name=f"idx{e}")
                        for g in range(8):
                            nc.sync.dma_start(idx_sb[16 * g:16 * (g + 1), :],
                                              lists_d[e])
                        idx_sbs.append(idx_sb)

            # ---- per-expert FFN ----
            gathered = {}

            def gathers(e):
                xg = xg_pool.tile([128, KT, C], BF16, tag="xg",
                                  name=f"xg{e}")
                nc.gpsimd.dma_gather(
                    out_ap=xg[:], in_ap=xbf[:], idxs_ap=idx_sbs[e][:],
                    num_idxs=C, num_idxs_reg=nir(e), elem_size=E, transpose=True)
                gt = gt_pool.tile([128, CT, 64], F32, tag="gt",
                                  name=f"gt{e}")
                nc.gpsimd.dma_gather(
                    out_ap=gt[:], in_ap=gat_d[:], idxs_ap=idx_sbs[e][:],
                    num_idxs=C, num_idxs_reg=nir(e), elem_size=64, transpose=False)
                gathered[e] = (xg, gt)

            gathers(0)
            gathers(1)
            load_w(1, 1)

            for e in range(NE):
                xg, gt = gathered.pop(e)

                hT = h_pool.tile([128, HT, C], BF16, tag="hT",
                                 name=f"hT{e}")
                for h in range(HT):
                    w1_sb = w_tiles[(e, h // (HT // 2))][0]
                    hh = h % (HT // 2)
                    for c0, cw in ((0, 512), (512, 128)):
                        ps = psH_pool.tile([128, cw], F32, tag="psH")
                        for k in range(KT):
                            nc.tensor.matmul(
                                ps[:],
                                lhsT=w1_sb[:, k, 128 * hh:128 * (hh + 1)],
                                rhs=xg[:, k, c0:c0 + cw],
                                start=(k == 0), stop=(k == KT - 1))
                        nc.scalar.activation(hT[:, h, c0:c0 + cw], ps[:],
                                             gelu_af, bias=b1_sb[:, e, h:h + 1])

                y_sb = y_pool.tile([128, CT, E], BF16, tag="y",
                                   name=f"y{e}")
                for tt in range(CT):
                    for n2 in range(2):
                        w2_sb = w_tiles[(e, n2)][1]
                        ps = psY_pool.tile([128, 512], F32, tag="psY")
                        for k2 in range(HT):
                            nc.tensor.matmul(
                                ps[:], lhsT=hT[:, k2, 128 * tt:128 * (tt + 1)],
                                rhs=w2_sb[:, k2, :],
                                start=(k2 == 0), stop=(k2 == HT - 1))
                        nc.scalar.activation(
                            y_sb[:, tt, 512 * n2:512 * (n2 + 1)], ps[:],
                            AF.Copy, scale=gt[:, tt, e:e + 1])

                nc.gpsimd.dma_scatter_add(
                    out_ap=out[:], in_ap=y_sb[:], idxs_ap=idx_sbs[e][:],
                    num_idxs=C, num_idxs_reg=nir(e), elem_size=E)

                w_tiles.pop((e, 0))
                w_tiles.pop((e, 1))
                if e + 2 < NE:
                    gathers(e + 2)
                    load_w(e + 2, 0)
                    load_w(e + 2, 1)

    return nc


def get_nc():
    if "nc" not in _CACHE:
        nc = _build_nc()
        nc.finalize()  # Bacc.compile(): reg alloc, library-load insertion, ...
        _CACHE["nc"] = nc
    return _CACHE["nc"]


def make_in_maps(inputs):
    x = np.asarray(inputs["x"], dtype=np.float32)
    Wr = np.asarray(inputs["Wr"], dtype=np.float32)
    br = np.asarray(inputs["br"], dtype=np.float32)
    W1 = np.asarray(inputs["W1"], dtype=np.float32)
    b1 = np.asarray(inputs["b1"], dtype=np.float32)
    W2 = np.asarray(inputs["W2"], dtype=np.float32)
    b2 = np.asarray(inputs["b2"], dtype=np.float32)
    assert x.shape == (B, N, E) and W1.shape == (NE, E, H) and W2.shape == (NE, H, E)
    if b2.any():
        raise NotImplementedError("nonzero b2 path not emitted in this kernel")

    # capacity guard: the kernel is compiled for a static per-expert capacity
    # of C tokens per core; verify the actual routing fits.
    logits = x.reshape(B * N, E) @ Wr + br
    part = np.partition(logits, NE - 2, axis=-1)[:, NE - 2:NE - 1]
    sel = logits >= part
    counts = sel.reshape(B, N, NE).sum(1)
    if counts.max() > C:
        raise RuntimeError(f"expert capacity exceeded: {counts.max()} > {C}")

    bf = ml_dtypes.bfloat16
    # p-major token ids: id(t) = (t%128)*16 + t//128, stored +1
    tok1 = (np.arange(128)[:, None] * 16 + np.arange(16)[None, :] + 1.0)
    tok1 = tok1.astype(np.float32).reshape(128, 16, 1)
    eye128 = np.eye(128, dtype=np.float32)
    brv = br.reshape(NE, 1).astype(np.float32)
    # b1v[p, e, h] = b1[e, h*128 + p]
    b1v = np.ascontiguousarray(b1.reshape(NE, HT, 128).transpose(2, 0, 1))
    # weights pre-transposed so each per-partition read is contiguous, and
    # split into halves (W1 by hidden range, W2 by output-column half):
    # w1[e, half, p, k*(H/2) + h] = W1[e, k*128+p, half*(H/2) + h]
    w1p = np.ascontiguousarray(
        W1.reshape(NE, KT, 128, 2, H // 2).transpose(0, 3, 2, 1, 4)
    ).reshape(NE, 2, 128, KT * (H // 2)).astype(bf)
    # w2[e, half, p, k2*(E/2) + f] = W2[e, k2*128+p, half*(E/2) + f]
    w2p = np.ascontiguousarray(
        W2.reshape(NE, HT, 128, 2, E // 2).transpose(0, 3, 2, 1, 4)
    ).reshape(NE, 2, 128, HT * (E // 2)).astype(bf)
    wrp = np.ascontiguousarray(
        Wr.reshape(KT, 128, NE).transpose(1, 0, 2)).reshape(128, KT * NE)

    in_maps = []
    for c in range(B):
        xc = x[c]
        # xT[p, k*N + n] = x[n, k*128+p]
        xTp = np.ascontiguousarray(
            xc.T.reshape(KT, 128, N).transpose(1, 0, 2)).reshape(128, KT * N)
        # xbf rows in id order (id = p*16 + bi -> token bi*128+p),
        # plus 128 zero dummy rows for the capacity-padding slots (id = N)
        xbfp = np.concatenate([
            np.ascontiguousarray(
                xc.reshape(16, 128, E).transpose(1, 0, 2)).reshape(N, E),
            np.zeros((NP - N, E), np.float32)], axis=0).astype(bf)
        in_maps.append({
            "xT": xTp,
            "xbf": xbfp,
            "wr": wrp,
            "w1": w1p,
            "w2": w2p,
            "tok1": tok1,
            "eye128": eye128,
            "brv": brv,
            "b1v": b1v,
        })
    return in_maps


def run(inputs, **kw):
    in_maps = make_in_maps(inputs)
    nc = get_nc()
    res = run_bass_kernel_spmd(nc, in_maps, list(range(B)), **kw)
    # undo the p-major row permutation: out_d[p*16+bi] = token bi*128+p
    out = np.stack(
        [np.asarray(res.results[c]["out"][0:N], dtype=np.float32)
         .reshape(128, 16, E).transpose(1, 0, 2).reshape(N, E)
         for c in range(B)],
        axis=0)
    return out, res


def kernel(**inputs):
    out, _ = run(inputs)
    return out
